# revision 1
# baseline (speedup 1.0000x reference)
"""DyGrEncoder (GatedGraphConv x3 + GRUCell + LSTM) as a Bass/Tile SPMD kernel
on 8 TRN2 NeuronCores — v2.

Key changes vs the v1 baseline:
- Gather via InstDMAGatherAnt (gpsimd `dma_gather`): batches of GATHER_N
  128-edge chunks per instruction instead of one indirect DMA per chunk,
  killing the ~1us/instruction SWDGE fixed cost (gpsimd was 65% busy).
  int16 indices force 4 source groups of 25000 m_full rows; edges are
  bucketed by (dst block, src group). Fallback GATHER_MODE=chunk uses the
  old per-chunk indirect DMA with no source grouping.
- S (scatter one-hot) matrices built SB chunks per DVE instruction via
  step-0 broadcast APs instead of one tensor_scalar per chunk.
- GRU matmuls in float32r (1 cycle/row at N>=256 vs 4 for fp32, ~1e-4 err).
- m-compute, bounce DMA and LSTM fused into the GRU chunk loop; the
  AllGather is issued as soon as the last bounce row is written.
"""
import os
import numpy as np
import ml_dtypes

import concourse.bass as bass
import concourse.mybir as mybir
import concourse.tile as tile
from concourse import bacc
from concourse.bass_utils import run_bass_kernel_spmd

P = 128
NCORES = 8
f32 = mybir.dt.float32
f32r = mybir.dt.float32r
bf16 = mybir.dt.bfloat16
i32 = mybir.dt.int32
i16 = mybir.dt.int16
AF = mybir.ActivationFunctionType
ALU = mybir.AluOpType
BF = ml_dtypes.bfloat16

SLAB_J = 8      # destination blocks per slab (msg/S tile granularity)
SB = 16         # chunks per batched S-build


# ----------------------------------------------------------------- host side

def _balance_nodes(dst, N, NL, NB):
    """Permute nodes so each of the 8*NB destination blocks holds 128 nodes
    whose total in-degree sits just under a multiple of 128. Returns newpos
    (orig id -> new id); new id = (core r, block j, slot) = r*NL + j*128 + s."""
    indeg = np.bincount(dst, minlength=N).astype(np.int64)
    order = np.argsort(-indeg, kind='stable')      # high degree first
    lastw = NL - (NB - 1) * P                      # slots in last position
    tail_n = lastw * NCORES                        # lowest-degree nodes there
    NBF = NB - 1                                   # full positions
    body = order[:N - tail_n]
    tail = order[N - tail_n:]
    E_body = int(indeg[body].sum())
    total_chunks = (E_body + 127) // 128

    q = total_chunks // (NBF * NCORES)             # per-block chunks target
    n_high = 0
    margin = 10
    sorted_deg = indeg[body]
    csum = np.concatenate([[0], np.cumsum(sorted_deg)])
    NBODY = len(body)
    while True:
        hi_bins = n_high * NCORES
        lo_bins = (NBF - n_high) * NCORES
        hi_nodes = hi_bins * P
        ok = True
        if hi_bins:
            t_hi = csum[hi_nodes]
            if t_hi / hi_bins > (q + 1) * P - margin:
                ok = False
        if lo_bins:
            t_lo = csum[NBODY] - csum[hi_nodes]
            if t_lo / lo_bins > q * P - margin:
                ok = False
        if ok or n_high >= NBF:
            break
        n_high += 1

    def snake(ids, nbins):
        k = len(ids) // nbins
        bins = [[] for _ in range(nbins)]
        pos = 0
        for rnd in range(k):
            idxs = range(nbins) if rnd % 2 == 0 else range(nbins - 1, -1, -1)
            for b in idxs:
                bins[b].append(ids[pos])
                pos += 1
        return bins

    hi_bins_n = n_high * NCORES
    hi_ids = body[:hi_bins_n * P]
    lo_ids = body[hi_bins_n * P:]
    bins = []
    if hi_bins_n:
        bins += snake(hi_ids, hi_bins_n)
    if NBF - n_high:
        bins += snake(lo_ids, (NBF - n_high) * NCORES)
    bins += snake(tail, NCORES)

    newpos = np.empty(N, dtype=np.int64)
    bi = 0
    for j in range(NB):
        for r in range(NCORES):
            ids = np.array(bins[bi])
            base = r * NL + j * P
            newpos[ids] = base + np.arange(len(ids))
            bi += 1
    return newpos


def _preprocess_edges(edge_index, edge_weight, N, NL, NB, G, GR):
    """Bucket each core's incoming edges by (dst block j, src group g); pad
    each bucket to cap[j,g]*128 edges (caps shared across cores, SPMD).

    Column layout: slabs of SLAB_J blocks; within a slab, groups are
    contiguous (g-major), blocks j-minor: (slab, g, j, k). Returns per-core
    tables:
      idx16: [128, ncols*8] int16, per-chunk 16-partition wrap, replicated
             to all 8 stripes (value = row within source group).
      esrc:  [128, ncols] int32 global rows (for the fallback path).
      eslot/ew: [128, ncols] bf16.
    """
    src = np.asarray(edge_index[0]).astype(np.int64)
    dst = np.asarray(edge_index[1]).astype(np.int64)
    w = np.asarray(edge_weight).astype(np.float32)

    per_core = []
    counts = np.zeros((NCORES, NB, G), dtype=np.int64)
    for r in range(NCORES):
        lo, hi = r * NL, (r + 1) * NL
        m = (dst >= lo) & (dst < hi)
        es, ed, ew = src[m], dst[m] - lo, w[m]
        g = (es % NL) // GR if G > 1 else np.zeros_like(es)
        jb = ed // P
        order = np.lexsort((es, g, jb))
        es, ed, ew, g, jb = (a[order] for a in (es, ed, ew, g, jb))
        for jj in range(NB):
            mj = jb == jj
            counts[r, jj] = np.bincount(g[mj], minlength=G)
        per_core.append((es, ed, ew, g, jb))

    cap = np.ceil(counts / 128).astype(np.int64).max(axis=0)   # [NB, G]
    cap = np.maximum(cap, 0)
    if G == 1:
        cap = np.maximum(cap, 1)

    # column layout
    slabs = []            # (j0, j1, col0, ncols_slab, {(j,g): colbase})
    colbase = {}
    c = 0
    for j0 in range(0, NB, SLAB_J):
        j1 = min(j0 + SLAB_J, NB)
        c0 = c
        for g in range(G):
            for j in range(j0, j1):
                colbase[(j, g)] = c
                c += int(cap[j, g])
        slabs.append((j0, j1, c0, c - c0))
    ncols = c

    out = []
    for r in range(NCORES):
        es, ed, ew_, g, jb = per_core[r]
        src_idx = np.zeros(ncols * 128, dtype=np.int32)
        slot = np.zeros(ncols * 128, dtype=np.float32)
        wgt = np.zeros(ncols * 128, dtype=np.float32)
        for jj in range(NB):
            for gg in range(G):
                m = (jb == jj) & (g == gg)
                cnt = int(m.sum())
                if cnt == 0:
                    continue
                pos = colbase[(jj, gg)] * 128
                src_idx[pos:pos + cnt] = es[m]
                slot[pos:pos + cnt] = (ed[m] - jj * P).astype(np.float32)
                wgt[pos:pos + cnt] = ew_[m]
        # int16 idx table: per-chunk wrap + 8-stripe replication
        rel = src_idx.reshape(ncols, 128)
        if G > 1:
            # row within group-q tensor: rank*GR + (local % GR)
            rel = (rel // NL) * GR + (rel % NL) % GR
        i16t = np.zeros((P, ncols * 8), dtype=np.int16)
        wrap = rel.reshape(ncols, 8, 16).astype(np.int16)   # [c, col, part]
        wrap = wrap.transpose(2, 0, 1).reshape(16, ncols * 8)
        for s in range(8):
            i16t[s * 16:(s + 1) * 16, :] = wrap
        out.append(dict(
            esrc=np.ascontiguousarray(src_idx.reshape(ncols, 128).T),
            idx16=i16t,
            eslot=np.ascontiguousarray(slot.reshape(ncols, 128).T).astype(BF),
            ew=np.ascontiguousarray(wgt.reshape(ncols, 128).T).astype(BF),
        ))
    return out, cap, ncols, slabs, colbase


def _padT(a, NLP, dt=np.float32):
    aT = np.ascontiguousarray(np.asarray(a).T.astype(np.float32))
    out = np.zeros((aT.shape[0], NLP), dtype=np.float32)
    out[:, :aT.shape[1]] = aT
    return out.astype(dt)


# ---------------------------------------------------------------- bass build

def _build(N, D, L, NL, NB, NLP, cap, ncols, slabs, colbase, G, GR,
           gather_n):
    nc = bacc.Bacc("TRN2", target_bir_lowering=False, debug=False,
                   num_devices=NCORES, dynamic_dma_scratch_size=32768)
    dp = nc.declare_dram_parameter

    hT0_in = dp("hT0", [P, NLP], f32r, isOutput=False)
    HT_in = dp("HT", [P, NLP], bf16, isOutput=False)
    CT_in = dp("CT", [P, NLP], f32, isOutput=False)
    convW_in = dp("convW", [P, L * P], f32r, isOutput=False)
    gWih_in = dp("gWihT", [P, 3 * P], f32r, isOutput=False)
    gWhh_in = dp("gWhhT", [P, 3 * P], f32r, isOutput=False)
    grub_in = dp("grub", [P, 4], f32, isOutput=False)
    lWih_in = dp("lWihT", [P, 4 * P], bf16, isOutput=False)
    lWhh_in = dp("lWhhT", [P, 4 * P], bf16, isOutput=False)
    lstmb_in = dp("lstmb", [P, 4], f32, isOutput=False)
    if G > 1:
        idx16_in = dp("idx16", [P, ncols * 8], i16, isOutput=False)
    else:
        esrc_in = dp("esrc", [P, ncols], i32, isOutput=False)
    eslot_in = dp("eslot", [P, ncols], bf16, isOutput=False)
    ew_in = dp("ew", [P, ncols], bf16, isOutput=False)
    iotaB_in = dp("iotaB", [P, SB * P], bf16, isOutput=False)
    Hout_ext = dp("HoutT", [P, NLP], f32, isOutput=True)
    Cout_ext = dp("CoutT", [P, NLP], f32, isOutput=True)

    lastw = NL - (NB - 1) * P          # valid rows in last (partial) block
    maxc = max(s[3] for s in slabs)

    # GRU chunks: (col_start, width, [blocks])
    chunks = []
    for s in range(0, NLP, 512):
        wdt = min(512, NLP - s)
        blks = list(range(s // P, min((s + wdt) // P, NB)))
        chunks.append((s, wdt, blks))

    with tile.TileContext(nc) as tc:
        with (
            tc.tile_pool(name="dram", bufs=1, space="DRAM") as dram,
            tc.tile_pool(name="persist", bufs=1) as pers,
            tc.tile_pool(name="msgp", bufs=8 if os.environ.get("GATHER_MODE", "ant") != "ant" else 2) as msgp,
            tc.tile_pool(name="sp", bufs=2) as sp,
            tc.tile_pool(name="sp0", bufs=1) as sp0,
            tc.tile_pool(name="aggp", bufs=2) as aggp,
            tc.tile_pool(name="mckp", bufs=2) as mckp,
            tc.tile_pool(name="tmp", bufs=1) as tp,
            tc.tile_pool(name="pagg", bufs=4, space="PSUM") as pagg,
            tc.tile_pool(name="pbig", bufs=4, space="PSUM") as pbig,
        ):
            # ---- persistent SBUF state
            hT = pers.tile([P, NLP], f32r, name="hT")
            convW = pers.tile([P, L * P], f32r, name="convW")
            gWih = pers.tile([P, 3 * P], f32r, name="gWih")
            gWhh = pers.tile([P, 3 * P], f32r, name="gWhh")
            grub = pers.tile([P, 4], f32, name="grub")
            lWih = pers.tile([P, 4 * P], bf16, name="lWih")
            lWhh = pers.tile([P, 4 * P], bf16, name="lWhh")
            lstmb = pers.tile([P, 4], f32, name="lstmb")
            if G > 1:
                idx16 = pers.tile([P, ncols * 8], i16, name="idx16")
            else:
                esrc = pers.tile([P, ncols], i32, name="esrc")
            eslot = pers.tile([P, ncols], bf16, name="eslot")
            ew = pers.tile([P, ncols], bf16, name="ew")
            iotaB = pers.tile([P, SB * P], bf16, name="iotaB")

            nc.sync.dma_start(hT[:], hT0_in[:])
            nc.sync.dma_start(convW[:], convW_in[:])
            nc.sync.dma_start(gWih[:], gWih_in[:])
            nc.sync.dma_start(gWhh[:], gWhh_in[:])
            nc.sync.dma_start(grub[:], grub_in[:])
            nc.sync.dma_start(lWih[:], lWih_in[:])
            nc.sync.dma_start(lWhh[:], lWhh_in[:])
            nc.sync.dma_start(lstmb[:], lstmb_in[:])
            if G > 1:
                nc.sync.dma_start(idx16[:], idx16_in[:])
            else:
                nc.sync.dma_start(esrc[:], esrc_in[:])
            nc.sync.dma_start(eslot[:], eslot_in[:])
            nc.sync.dma_start(ew[:], ew_in[:])
            nc.sync.dma_start(iotaB[:], iotaB_in[:])

            m_bounce = [dram.tile([NL, P], bf16, name=f"mb{l}")
                        for l in range(L)]
            if G > 1:
                m_full = [[dram.tile([NCORES * GR, P], bf16,
                                     name=f"mf{l}q{q}", addr_space="Shared")
                           for q in range(G)] for l in range(L)]
            else:
                m_full = [dram.tile([N, P], bf16, name=f"mf{l}",
                                    addr_space="Shared")
                          for l in range(L)]

            def m_chunk_and_dma(l, ci):
                """m tiles for GRU chunk ci of layer l + bounce DMA + AG."""
                s, wdt, blks = chunks[ci]
                mck = mckp.tile([P, 512], bf16, name="mck", tag="mck")
                for ti, j in enumerate(blks):
                    pm = pagg.tile([P, P], f32, name="pm", tag="agg128")
                    nc.tensor.matmul(pm[:],
                                     lhsT=hT[:, j * P:(j + 1) * P],
                                     rhs=convW[:, l * P:(l + 1) * P],
                                     start=True, stop=True)
                    nc.scalar.copy(out=mck[:, ti * P:(ti + 1) * P], in_=pm[:])
                row0 = s
                nfull = len(blks) if blks[-1] != NB - 1 else len(blks) - 1
                if nfull:
                    nc.sync.dma_start(
                        m_bounce[l][row0:row0 + nfull * P, :].rearrange(
                            "(t p) f -> p t f", p=P),
                        mck[:, :nfull * P].rearrange("p (t f) -> p t f", f=P))
                if blks[-1] == NB - 1:
                    r0 = (NB - 1) * P
                    nc.sync.dma_start(
                        m_bounce[l][r0:r0 + lastw, :],
                        mck[:lastw, nfull * P:(nfull + 1) * P])
                if G > 1:
                    # fire AG piece q once its last local row is written
                    lastrow = min(s + wdt, NL) - 1
                    for q in range(G):
                        hi = (q + 1) * GR - 1
                        if s <= hi <= lastrow:
                            nc.gpsimd.collective_compute(
                                "AllGather", ALU.bypass,
                                replica_groups=[list(range(NCORES))],
                                ins=[m_bounce[l][q * GR:(q + 1) * GR,
                                                 :].opt()],
                                outs=[m_full[l][q][:].opt()])
                elif ci == len(chunks) - 1:
                    nc.gpsimd.collective_compute(
                        "AllGather", ALU.bypass,
                        replica_groups=[list(range(NCORES))],
                        ins=[m_bounce[l][:].opt()],
                        outs=[m_full[l][:].opt()])

            # ---- layer 0 m: from initial hT
            for ci in range(len(chunks)):
                m_chunk_and_dma(0, ci)

            for l in range(L):
                slab_tiles = {}

                def ensure_slab(si):
                    if si in slab_tiles:
                        return slab_tiles[si]
                    j0, j1, c0, k = slabs[si]
                    msg = None
                    if G > 1:
                        msg = msgp.tile([P, maxc * P], bf16, name="msg",
                                        tag="msg")
                        # per-(g) runs of consecutive chunks, split by
                        # gather_n
                        for g in range(G):
                            g0 = colbase[(j0, g)]
                            gcols = sum(int(cap[j, g])
                                        for j in range(j0, j1))
                            cc = g0
                            while cc < g0 + gcols:
                                kk = min(gather_n, g0 + gcols - cc)
                                nc.gpsimd.dma_gather(
                                    out_ap=msg[:, (cc - c0) * P:
                                               (cc - c0 + kk) * P].rearrange(
                                        "p (c f) -> p c f", f=P),
                                    in_ap=m_full[l][g][:],
                                    idxs_ap=idx16[:, cc * 8:(cc + kk) * 8],
                                    num_idxs=kk * 128,
                                    num_idxs_reg=kk * 128,
                                    elem_size=P,
                                    single_packet=False)
                                cc += kk
                    # G == 1: per-chunk gathers are issued in the block loop
                    # right before their matmul (keeps SWDGE rings drained).
                    S = sp.tile([P, maxc * P], bf16, name="S", tag="S")
                    for b0 in range(0, k, SB):
                        nb = min(SB, k - b0)
                        g0c = c0 + b0
                        t_ = sp0.tile([P, SB * P], bf16, name="S0", tag="S0")
                        nc.vector.tensor_tensor(
                            out=t_[:, :nb * P].rearrange(
                                "p (c f) -> p c f", f=P),
                            in0=iotaB[:, :nb * P].rearrange(
                                "p (c f) -> p c f", f=P),
                            in1=eslot[:, g0c:g0c + nb].to_broadcast(
                                [P, nb, P]),
                            op=ALU.is_equal)
                        nc.vector.tensor_tensor(
                            out=S[:, b0 * P:(b0 + nb) * P].rearrange(
                                "p (c f) -> p c f", f=P),
                            in0=t_[:, :nb * P].rearrange(
                                "p (c f) -> p c f", f=P),
                            in1=ew[:, g0c:g0c + nb].to_broadcast([P, nb, P]),
                            op=ALU.mult)
                    slab_tiles[si] = (msg, S, c0)
                    return slab_tiles[si]

                for ci, (s, wdt, blks) in enumerate(chunks):
                    agg = aggp.tile([P, 512], f32r, name="agg", tag="agg")
                    for bi, j in enumerate(blks):
                        si = j // SLAB_J
                        msg, S, c0 = ensure_slab(si)
                        pj = pagg.tile([P, P], f32, name="pj", tag="agg128")
                        mm = []
                        for g in range(G):
                            for k in range(int(cap[j, g])):
                                mm.append(colbase[(j, g)] + k)
                        if not mm:
                            nc.vector.memset(pj[:], 0.0)
                        for ki, cc in enumerate(mm):
                            if G > 1:
                                lhs = msg[:, (cc - c0) * P:(cc - c0 + 1) * P]
                            else:
                                mt = msgp.tile([P, P], bf16, name="mt",
                                               tag="mt")
                                nc.gpsimd.indirect_dma_start(
                                    out=mt[:], out_offset=None,
                                    in_=m_full[l][:],
                                    in_offset=bass.IndirectOffsetOnAxis(
                                        ap=esrc[:, cc:cc + 1], axis=0))
                                lhs = mt[:]
                            nc.tensor.matmul(
                                pj[:],
                                lhsT=lhs,
                                rhs=S[:, (cc - c0) * P:(cc - c0 + 1) * P],
                                start=(ki == 0),
                                stop=(ki == len(mm) - 1))
                        nc.scalar.copy(out=agg[:, bi * P:(bi + 1) * P],
                                       in_=pj[:])

                    # ---- GRU for this chunk (f32r matmuls)
                    sl = slice(s, s + wdt)
                    pr = pbig.tile([P, 512], f32, name="pr", tag="big")
                    pz = pbig.tile([P, 512], f32, name="pz", tag="big")
                    pin = pbig.tile([P, 512], f32, name="pin", tag="big")
                    phn = pbig.tile([P, 512], f32, name="phn", tag="big")
                    for (ps_, g) in ((pr, 0), (pz, 1)):
                        gs = slice(g * P, (g + 1) * P)
                        nc.tensor.matmul(ps_[:, :wdt], lhsT=gWih[:, gs],
                                         rhs=agg[:, :wdt],
                                         start=True, stop=False)
                        nc.tensor.matmul(ps_[:, :wdt], lhsT=gWhh[:, gs],
                                         rhs=hT[:, sl],
                                         start=False, stop=True)
                    gn = slice(2 * P, 3 * P)
                    nc.tensor.matmul(pin[:, :wdt], lhsT=gWih[:, gn],
                                     rhs=agg[:, :wdt], start=True, stop=True)
                    nc.tensor.matmul(phn[:, :wdt], lhsT=gWhh[:, gn],
                                     rhs=hT[:, sl], start=True, stop=True)

                    rt = tp.tile([P, 512], f32, name="rt", tag="ew1")
                    zt = tp.tile([P, 512], f32, name="zt", tag="ew2")
                    t2 = tp.tile([P, 512], f32, name="t2", tag="ew3")
                    t3 = tp.tile([P, 512], f32, name="t3", tag="ew4")
                    nt = tp.tile([P, 512], f32, name="nt", tag="ew5")
                    dd = tp.tile([P, 512], f32, name="dd", tag="ew6")
                    ee = tp.tile([P, 512], f32, name="ee", tag="ew7")
                    nc.scalar.activation(rt[:, :wdt], pr[:, :wdt],
                                         AF.Sigmoid, bias=grub[:, 0:1])
                    nc.scalar.activation(zt[:, :wdt], pz[:, :wdt],
                                         AF.Sigmoid, bias=grub[:, 1:2])
                    nc.vector.scalar_tensor_tensor(
                        out=t2[:, :wdt], in0=phn[:, :wdt],
                        scalar=grub[:, 3:4], in1=rt[:, :wdt],
                        op0=ALU.add, op1=ALU.mult)
                    nc.vector.tensor_add(t3[:, :wdt], t2[:, :wdt],
                                         pin[:, :wdt])
                    nc.scalar.activation(nt[:, :wdt], t3[:, :wdt],
                                         AF.Tanh, bias=grub[:, 2:3])
                    hTf = hT[:, sl].bitcast(f32)
                    nc.vector.tensor_sub(dd[:, :wdt], hTf, nt[:, :wdt])
                    nc.vector.tensor_mul(ee[:, :wdt], zt[:, :wdt],
                                         dd[:, :wdt])
                    nc.vector.tensor_add(hT[:, sl], nt[:, :wdt],
                                         ee[:, :wdt])

                    if l < L - 1:
                        m_chunk_and_dma(l + 1, ci)
                    else:
                        # ---- LSTM for this chunk
                        hx = tp.tile([P, 512], bf16, name="hx", tag="ewx")
                        nc.vector.tensor_copy(hx[:, :wdt], hTf)
                        ht = tp.tile([P, 512], bf16, name="htc", tag="ewhl")
                        ct = tp.tile([P, 512], f32, name="ctc", tag="ewcl")
                        nc.sync.dma_start(ht[:, :wdt], HT_in[:, sl])
                        nc.sync.dma_start(ct[:, :wdt], CT_in[:, sl])
                        pg = [pbig.tile([P, 512], f32, name=f"pl{g}",
                                        tag="big") for g in range(4)]
                        for g in range(4):
                            gs = slice(g * P, (g + 1) * P)
                            nc.tensor.matmul(pg[g][:, :wdt],
                                             lhsT=lWih[:, gs],
                                             rhs=hx[:, :wdt], start=True,
                                             stop=False)
                            nc.tensor.matmul(pg[g][:, :wdt],
                                             lhsT=lWhh[:, gs],
                                             rhs=ht[:, :wdt], start=False,
                                             stop=True)
                        it = tp.tile([P, 512], f32, name="it", tag="ew1")
                        ft = tp.tile([P, 512], f32, name="ft", tag="ew2")
                        gt = tp.tile([P, 512], f32, name="gt", tag="ew3")
                        ot = tp.tile([P, 512], f32, name="ot", tag="ew4")
                        nc.scalar.activation(it[:, :wdt], pg[0][:, :wdt],
                                             AF.Sigmoid, bias=lstmb[:, 0:1])
                        nc.scalar.activation(ft[:, :wdt], pg[1][:, :wdt],
                                             AF.Sigmoid, bias=lstmb[:, 1:2])
                        nc.scalar.activation(gt[:, :wdt], pg[2][:, :wdt],
                                             AF.Tanh, bias=lstmb[:, 2:3])
                        nc.scalar.activation(ot[:, :wdt], pg[3][:, :wdt],
                                             AF.Sigmoid, bias=lstmb[:, 3:4])
                        t1 = tp.tile([P, 512], f32, name="lt1", tag="ew5")
                        t2b = tp.tile([P, 512], f32, name="lt2", tag="ew6")
                        cn = tp.tile([P, 512], f32, name="cn", tag="ew7")
                        tc_ = tp.tile([P, 512], f32, name="tcx", tag="ewt")
                        hn = tp.tile([P, 512], f32, name="hn", tag="ewh")
                        nc.vector.tensor_mul(t1[:, :wdt], ft[:, :wdt],
                                             ct[:, :wdt])
                        nc.vector.tensor_mul(t2b[:, :wdt], it[:, :wdt],
                                             gt[:, :wdt])
                        nc.vector.tensor_add(cn[:, :wdt], t1[:, :wdt],
                                             t2b[:, :wdt])
                        nc.scalar.activation(tc_[:, :wdt], cn[:, :wdt],
                                             AF.Tanh)
                        nc.vector.tensor_mul(hn[:, :wdt], ot[:, :wdt],
                                             tc_[:, :wdt])
                        nc.sync.dma_start(Cout_ext[:, sl], cn[:, :wdt])
                        nc.sync.dma_start(Hout_ext[:, sl], hn[:, :wdt])
    return nc


_CACHE = {}


def kernel(X, edge_index, edge_weight, H, C, conv_W,
           gru_Wih, gru_Whh, gru_bih, gru_bhh,
           lstm_Wih, lstm_Whh, lstm_bih, lstm_bhh):
    X = np.asarray(X, dtype=np.float32)
    H = np.asarray(H, dtype=np.float32)
    C = np.asarray(C, dtype=np.float32)
    conv_W = np.asarray(conv_W, dtype=np.float32)
    edge_index = np.asarray(edge_index)
    edge_weight = np.asarray(edge_weight, dtype=np.float32)

    N, D = X.shape
    L = conv_W.shape[0]
    assert D == P and N % NCORES == 0
    NL = N // NCORES
    NB = (NL + P - 1) // P
    NLP = NB * P

    mode = os.environ.get("GATHER_MODE", "ant")
    G = 4 if mode == "ant" else 1
    GR = (NL // 4) if G > 1 else N
    gather_n = int(os.environ.get("GATHER_N", "16"))

    src = edge_index[0].astype(np.int64)
    dst = edge_index[1].astype(np.int64)
    newpos = _balance_nodes(dst, N, NL, NB)
    perm = np.empty(N, dtype=np.int64)          # new id -> orig id
    perm[newpos] = np.arange(N)
    e_new = np.stack([newpos[src], newpos[dst]])

    edata, cap, ncols, slabs, colbase = _preprocess_edges(
        e_new, edge_weight, N, NL, NB, G, GR)

    key = (N, D, L, ncols, cap.tobytes(), G, gather_n)
    if key not in _CACHE:
        nc = _build(N, D, L, NL, NB, NLP, cap, ncols, slabs, colbase, G, GR,
                    gather_n)
        nc.compile()
        _CACHE[key] = nc
    nc = _CACHE[key]

    Xp, Hp, Cp = X[perm], H[perm], C[perm]

    gWihT = np.ascontiguousarray(np.asarray(gru_Wih, np.float32).T)
    gWhhT = np.ascontiguousarray(np.asarray(gru_Whh, np.float32).T)
    lWihT = np.ascontiguousarray(
        np.asarray(lstm_Wih, np.float32).T).astype(BF)
    lWhhT = np.ascontiguousarray(
        np.asarray(lstm_Whh, np.float32).T).astype(BF)
    gb = np.asarray(gru_bih, np.float32)
    gb2 = np.asarray(gru_bhh, np.float32)
    grub = np.stack([gb[0:D] + gb2[0:D], gb[D:2 * D] + gb2[D:2 * D],
                     gb[2 * D:3 * D], gb2[2 * D:3 * D]], axis=1)
    lb = np.asarray(lstm_bih, np.float32) + np.asarray(lstm_bhh, np.float32)
    lstmb = np.stack([lb[g * D:(g + 1) * D] for g in range(4)], axis=1)
    iotaB = np.ascontiguousarray(np.broadcast_to(
        np.arange(P, dtype=np.float32), (P, SB, P)).reshape(P, SB * P)
    ).astype(BF)
    convWb = np.ascontiguousarray(
        np.concatenate([conv_W[i] for i in range(L)], axis=1))

    in_maps = []
    for r in range(NCORES):
        sl = slice(r * NL, (r + 1) * NL)
        im = dict(
            hT0=_padT(Xp[sl], NLP),
            HT=_padT(Hp[sl], NLP, BF),
            CT=_padT(Cp[sl], NLP),
            convW=convWb, gWihT=gWihT, gWhhT=gWhhT, grub=grub,
            lWihT=lWihT, lWhhT=lWhhT, lstmb=lstmb,
            eslot=edata[r]['eslot'], ew=edata[r]['ew'],
            iotaB=iotaB,
        )
        if G > 1:
            im['idx16'] = edata[r]['idx16']
        else:
            im['esrc'] = edata[r]['esrc']
        in_maps.append(im)

    if os.environ.get("KERNEL_SIM"):
        from concourse import bass_interp
        simu = bass_interp.MultiCoreSim(nc, NCORES)
        for r in range(NCORES):
            for k, v in in_maps[r].items():
                simu.cores[r].tensor(k)[:] = v
        simu.simulate()
        results = [{k: np.asarray(simu.cores[r].mem_tensor(k))
                    for k in ("HoutT", "CoutT")} for r in range(NCORES)]
    else:
        trace = bool(int(os.environ.get("KERNEL_TRACE", "0")))
        res = run_bass_kernel_spmd(nc, in_maps, core_ids=list(range(NCORES)),
                                   trace=trace)
        if trace:
            kernel.last_exec_time_ns = res.exec_time_ns
        results = res.results

    Hnew = np.empty((N, D), dtype=np.float32)
    Cnew = np.empty((N, D), dtype=np.float32)
    for r in range(NCORES):
        sl = slice(r * NL, (r + 1) * NL)
        Hnew[sl] = results[r]["HoutT"].T[:NL]
        Cnew[sl] = results[r]["CoutT"].T[:NL]
    Hout = Hnew[newpos]
    Cout = Cnew[newpos]
    return Hout, Hout, Cout


kernel.last_exec_time_ns = None



# revision 3
# speedup vs baseline: 1.4556x; 1.4556x over previous
"""DyGrEncoder (GatedGraphConv x3 + GRUCell + LSTM) as a Bass/Tile SPMD kernel
on 8 TRN2 NeuronCores — v2.

Key changes vs the v1 baseline:
- Gather via InstDMAGatherAnt (gpsimd `dma_gather`): batches of GATHER_N
  128-edge chunks per instruction instead of one indirect DMA per chunk,
  killing the ~1us/instruction SWDGE fixed cost (gpsimd was 65% busy).
  int16 indices force 4 source groups of 25000 m_full rows; edges are
  bucketed by (dst block, src group). Fallback GATHER_MODE=chunk uses the
  old per-chunk indirect DMA with no source grouping.
- S (scatter one-hot) matrices built SB chunks per DVE instruction via
  step-0 broadcast APs instead of one tensor_scalar per chunk.
- GRU matmuls in float32r (1 cycle/row at N>=256 vs 4 for fp32, ~1e-4 err).
- m-compute, bounce DMA and LSTM fused into the GRU chunk loop; the
  AllGather is issued as soon as the last bounce row is written.
"""
import os
import numpy as np
import ml_dtypes

import concourse.bass as bass
import concourse.mybir as mybir
import concourse.tile as tile
from concourse import bacc
from concourse.bass_utils import run_bass_kernel_spmd

P = 128
NCORES = 8
f32 = mybir.dt.float32
f32r = mybir.dt.float32r
bf16 = mybir.dt.bfloat16
i32 = mybir.dt.int32
i16 = mybir.dt.int16
AF = mybir.ActivationFunctionType
ALU = mybir.AluOpType
BF = ml_dtypes.bfloat16

SLAB_J = 8      # destination blocks per slab (msg/S tile granularity)
SB = 16         # chunks per batched S-build


# ----------------------------------------------------------------- host side

def _balance_nodes(dst, N, NL, NB):
    """Permute nodes so each of the 8*NB destination blocks holds 128 nodes
    whose total in-degree sits just under a multiple of 128. Returns newpos
    (orig id -> new id); new id = (core r, block j, slot) = r*NL + j*128 + s."""
    indeg = np.bincount(dst, minlength=N).astype(np.int64)
    order = np.argsort(-indeg, kind='stable')      # high degree first
    lastw = NL - (NB - 1) * P                      # slots in last position
    tail_n = lastw * NCORES                        # lowest-degree nodes there
    NBF = NB - 1                                   # full positions
    body = order[:N - tail_n]
    tail = order[N - tail_n:]
    E_body = int(indeg[body].sum())
    total_chunks = (E_body + 127) // 128

    q = total_chunks // (NBF * NCORES)             # per-block chunks target
    n_high = 0
    margin = 10
    sorted_deg = indeg[body]
    csum = np.concatenate([[0], np.cumsum(sorted_deg)])
    NBODY = len(body)
    while True:
        hi_bins = n_high * NCORES
        lo_bins = (NBF - n_high) * NCORES
        hi_nodes = hi_bins * P
        ok = True
        if hi_bins:
            t_hi = csum[hi_nodes]
            if t_hi / hi_bins > (q + 1) * P - margin:
                ok = False
        if lo_bins:
            t_lo = csum[NBODY] - csum[hi_nodes]
            if t_lo / lo_bins > q * P - margin:
                ok = False
        if ok or n_high >= NBF:
            break
        n_high += 1

    def snake(ids, nbins):
        k = len(ids) // nbins
        bins = [[] for _ in range(nbins)]
        pos = 0
        for rnd in range(k):
            idxs = range(nbins) if rnd % 2 == 0 else range(nbins - 1, -1, -1)
            for b in idxs:
                bins[b].append(ids[pos])
                pos += 1
        return bins

    hi_bins_n = n_high * NCORES
    hi_ids = body[:hi_bins_n * P]
    lo_ids = body[hi_bins_n * P:]
    bins = []
    if hi_bins_n:
        bins += snake(hi_ids, hi_bins_n)
    if NBF - n_high:
        bins += snake(lo_ids, (NBF - n_high) * NCORES)
    bins += snake(tail, NCORES)

    newpos = np.empty(N, dtype=np.int64)
    bi = 0
    for j in range(NB):
        for r in range(NCORES):
            ids = np.array(bins[bi])
            base = r * NL + j * P
            newpos[ids] = base + np.arange(len(ids))
            bi += 1
    return newpos


def _preprocess_edges(edge_index, edge_weight, N, NL, NB, G, GR):
    """Bucket each core's incoming edges by (dst block j, src group g); pad
    each bucket to cap[j,g]*128 edges (caps shared across cores, SPMD).

    Column layout: slabs of SLAB_J blocks; within a slab, groups are
    contiguous (g-major), blocks j-minor: (slab, g, j, k). Returns per-core
    tables:
      idx16: [128, ncols*8] int16, per-chunk 16-partition wrap, replicated
             to all 8 stripes (value = row within source group).
      esrc:  [128, ncols] int32 global rows (for the fallback path).
      eslot/ew: [128, ncols] bf16.
    """
    src = np.asarray(edge_index[0]).astype(np.int64)
    dst = np.asarray(edge_index[1]).astype(np.int64)
    w = np.asarray(edge_weight).astype(np.float32)

    per_core = []
    counts = np.zeros((NCORES, NB, G), dtype=np.int64)
    for r in range(NCORES):
        lo, hi = r * NL, (r + 1) * NL
        m = (dst >= lo) & (dst < hi)
        es, ed, ew = src[m], dst[m] - lo, w[m]
        g = (es % NL) // GR if G > 1 else np.zeros_like(es)
        jb = ed // P
        order = np.lexsort((es, g, jb))
        es, ed, ew, g, jb = (a[order] for a in (es, ed, ew, g, jb))
        for jj in range(NB):
            mj = jb == jj
            counts[r, jj] = np.bincount(g[mj], minlength=G)
        per_core.append((es, ed, ew, g, jb))

    cap = np.ceil(counts / 128).astype(np.int64).max(axis=0)   # [NB, G]
    cap = np.maximum(cap, 0)
    if G == 1:
        cap = np.maximum(cap, 1)

    # column layout
    slabs = []            # (j0, j1, col0, ncols_slab, {(j,g): colbase})
    colbase = {}
    c = 0
    for j0 in range(0, NB, SLAB_J):
        j1 = min(j0 + SLAB_J, NB)
        c0 = c
        for g in range(G):
            for j in range(j0, j1):
                colbase[(j, g)] = c
                c += int(cap[j, g])
        slabs.append((j0, j1, c0, c - c0))
    ncols = c

    out = []
    for r in range(NCORES):
        es, ed, ew_, g, jb = per_core[r]
        src_idx = np.zeros(ncols * 128, dtype=np.int32)
        slot = np.zeros(ncols * 128, dtype=np.float32)
        wgt = np.zeros(ncols * 128, dtype=np.float32)
        for jj in range(NB):
            for gg in range(G):
                m = (jb == jj) & (g == gg)
                cnt = int(m.sum())
                if cnt == 0:
                    continue
                pos = colbase[(jj, gg)] * 128
                src_idx[pos:pos + cnt] = es[m]
                slot[pos:pos + cnt] = (ed[m] - jj * P).astype(np.float32)
                wgt[pos:pos + cnt] = ew_[m]
        # int16 idx table: per-chunk wrap + 8-stripe replication
        rel = src_idx.reshape(ncols, 128)
        if G > 1:
            # row within group-q tensor: rank*GR + (local % GR)
            rel = (rel // NL) * GR + (rel % NL) % GR
        i16t = np.zeros((P, ncols * 8), dtype=np.int16)
        wrap = rel.reshape(ncols, 8, 16).astype(np.int16)   # [c, col, part]
        wrap = wrap.transpose(2, 0, 1).reshape(16, ncols * 8)
        for s in range(8):
            i16t[s * 16:(s + 1) * 16, :] = wrap
        out.append(dict(
            esrc=np.ascontiguousarray(src_idx.reshape(ncols, 128).T),
            idx16=i16t,
            eslot=np.ascontiguousarray(slot.reshape(ncols, 128).T).astype(BF),
            ew=np.ascontiguousarray(wgt.reshape(ncols, 128).T).astype(BF),
        ))
    return out, cap, ncols, slabs, colbase


def _padT(a, NLP, dt=np.float32):
    aT = np.ascontiguousarray(np.asarray(a).T.astype(np.float32))
    out = np.zeros((aT.shape[0], NLP), dtype=np.float32)
    out[:, :aT.shape[1]] = aT
    return out.astype(dt)


# ---------------------------------------------------------------- bass build

def _build(N, D, L, NL, NB, NLP, cap, ncols, slabs, colbase, G, GR,
           gather_n):
    nc = bacc.Bacc("TRN2", target_bir_lowering=False, debug=False,
                   num_devices=NCORES, dynamic_dma_scratch_size=32768,
                   num_swdge_queues=4)
    dp = nc.declare_dram_parameter

    hT0_in = dp("hT0", [P, NLP], f32r, isOutput=False)
    HT_in = dp("HT", [P, NLP], bf16, isOutput=False)
    CT_in = dp("CT", [P, NLP], f32, isOutput=False)
    convW_in = dp("convW", [P, L * P], f32r, isOutput=False)
    gWih_in = dp("gWihT", [P, 3 * P], f32r, isOutput=False)
    gWhh_in = dp("gWhhT", [P, 3 * P], f32r, isOutput=False)
    grub_in = dp("grub", [P, 4], f32, isOutput=False)
    lWih_in = dp("lWihT", [P, 4 * P], bf16, isOutput=False)
    lWhh_in = dp("lWhhT", [P, 4 * P], bf16, isOutput=False)
    lstmb_in = dp("lstmb", [P, 4], f32, isOutput=False)
    if G > 1:
        idx16_in = dp("idx16", [P, ncols * 8], i16, isOutput=False)
    else:
        esrc_in = dp("esrc", [P, ncols], i32, isOutput=False)
    eslot_in = dp("eslot", [P, ncols], bf16, isOutput=False)
    ew_in = dp("ew", [P, ncols], bf16, isOutput=False)
    iotaB_in = dp("iotaB", [P, SB * P], bf16, isOutput=False)
    Hout_ext = dp("HoutT", [P, NLP], f32, isOutput=True)
    Cout_ext = dp("CoutT", [P, NLP], f32, isOutput=True)

    lastw = NL - (NB - 1) * P          # valid rows in last (partial) block
    maxc = max(s[3] for s in slabs)

    # GRU chunks: (col_start, width, [blocks])
    chunks = []
    for s in range(0, NLP, 512):
        wdt = min(512, NLP - s)
        blks = list(range(s // P, min((s + wdt) // P, NB)))
        chunks.append((s, wdt, blks))

    with tile.TileContext(nc) as tc:
        with (
            tc.tile_pool(name="dram", bufs=1, space="DRAM") as dram,
            tc.tile_pool(name="persist", bufs=1) as pers,
            tc.tile_pool(name="msgp", bufs=8 if os.environ.get("GATHER_MODE", "ant") != "ant" else 2) as msgp,
            tc.tile_pool(name="sp", bufs=2) as sp,
            tc.tile_pool(name="sp0", bufs=1) as sp0,
            tc.tile_pool(name="aggp", bufs=2) as aggp,
            tc.tile_pool(name="mckp", bufs=2) as mckp,
            tc.tile_pool(name="tmp", bufs=1) as tp,
            tc.tile_pool(name="pagg", bufs=4, space="PSUM") as pagg,
            tc.tile_pool(name="pbig", bufs=4, space="PSUM") as pbig,
        ):
            # ---- persistent SBUF state
            hT = pers.tile([P, NLP], f32r, name="hT")
            convW = pers.tile([P, L * P], f32r, name="convW")
            gWih = pers.tile([P, 3 * P], f32r, name="gWih")
            gWhh = pers.tile([P, 3 * P], f32r, name="gWhh")
            grub = pers.tile([P, 4], f32, name="grub")
            lWih = pers.tile([P, 4 * P], bf16, name="lWih")
            lWhh = pers.tile([P, 4 * P], bf16, name="lWhh")
            lstmb = pers.tile([P, 4], f32, name="lstmb")
            if G > 1:
                idx16 = pers.tile([P, ncols * 8], i16, name="idx16")
            else:
                esrc = pers.tile([P, ncols], i32, name="esrc")
            eslot = pers.tile([P, ncols], bf16, name="eslot")
            ew = pers.tile([P, ncols], bf16, name="ew")
            iotaB = pers.tile([P, SB * P], bf16, name="iotaB")

            nc.sync.dma_start(hT[:], hT0_in[:])
            nc.sync.dma_start(convW[:], convW_in[:])
            nc.sync.dma_start(gWih[:], gWih_in[:])
            nc.sync.dma_start(gWhh[:], gWhh_in[:])
            nc.sync.dma_start(grub[:], grub_in[:])
            nc.sync.dma_start(lWih[:], lWih_in[:])
            nc.sync.dma_start(lWhh[:], lWhh_in[:])
            nc.sync.dma_start(lstmb[:], lstmb_in[:])
            if G > 1:
                nc.sync.dma_start(idx16[:], idx16_in[:])
            else:
                nc.sync.dma_start(esrc[:], esrc_in[:])
            nc.sync.dma_start(eslot[:], eslot_in[:])
            nc.sync.dma_start(ew[:], ew_in[:])
            nc.sync.dma_start(iotaB[:], iotaB_in[:])

            m_bounce = [dram.tile([NL, P], bf16, name=f"mb{l}")
                        for l in range(L)]
            if G > 1:
                m_full = [[dram.tile([NCORES * GR, P], bf16,
                                     name=f"mf{l}q{q}", addr_space="Shared")
                           for q in range(G)] for l in range(L)]
            else:
                m_full = [dram.tile([N, P], bf16, name=f"mf{l}",
                                    addr_space="Shared")
                          for l in range(L)]

            def m_chunk_and_dma(l, ci):
                """m tiles for GRU chunk ci of layer l + bounce DMA + AG."""
                s, wdt, blks = chunks[ci]
                mck = mckp.tile([P, 512], bf16, name="mck", tag="mck")
                for ti, j in enumerate(blks):
                    pm = pagg.tile([P, P], f32, name="pm", tag="agg128")
                    nc.tensor.matmul(pm[:],
                                     lhsT=hT[:, j * P:(j + 1) * P],
                                     rhs=convW[:, l * P:(l + 1) * P],
                                     start=True, stop=True)
                    nc.scalar.copy(out=mck[:, ti * P:(ti + 1) * P], in_=pm[:])
                row0 = s
                nfull = len(blks) if blks[-1] != NB - 1 else len(blks) - 1
                if nfull:
                    nc.sync.dma_start(
                        m_bounce[l][row0:row0 + nfull * P, :].rearrange(
                            "(t p) f -> p t f", p=P),
                        mck[:, :nfull * P].rearrange("p (t f) -> p t f", f=P))
                if blks[-1] == NB - 1:
                    r0 = (NB - 1) * P
                    nc.sync.dma_start(
                        m_bounce[l][r0:r0 + lastw, :],
                        mck[:lastw, nfull * P:(nfull + 1) * P])
                if G > 1:
                    # fire AG piece q once its last local row is written
                    lastrow = min(s + wdt, NL) - 1
                    for q in range(G):
                        hi = (q + 1) * GR - 1
                        if s <= hi <= lastrow:
                            nc.gpsimd.collective_compute(
                                "AllGather", ALU.bypass,
                                replica_groups=[list(range(NCORES))],
                                ins=[m_bounce[l][q * GR:(q + 1) * GR,
                                                 :].opt()],
                                outs=[m_full[l][q][:].opt()])
                elif ci == len(chunks) - 1:
                    nc.gpsimd.collective_compute(
                        "AllGather", ALU.bypass,
                        replica_groups=[list(range(NCORES))],
                        ins=[m_bounce[l][:].opt()],
                        outs=[m_full[l][:].opt()])

            # ---- layer 0 m: from initial hT
            for ci in range(len(chunks)):
                m_chunk_and_dma(0, ci)

            for l in range(L):
                slab_tiles = {}

                def ensure_slab(si):
                    if si in slab_tiles:
                        return slab_tiles[si]
                    j0, j1, c0, k = slabs[si]
                    msg = None
                    if G > 1:
                        msg = msgp.tile([P, maxc * P], bf16, name="msg",
                                        tag="msg")
                        # per-(g) runs of consecutive chunks, split by
                        # gather_n
                        for g in range(G):
                            g0 = colbase[(j0, g)]
                            gcols = sum(int(cap[j, g])
                                        for j in range(j0, j1))
                            cc = g0
                            while cc < g0 + gcols:
                                kk = min(gather_n, g0 + gcols - cc)
                                nc.gpsimd.dma_gather(
                                    out_ap=msg[:, (cc - c0) * P:
                                               (cc - c0 + kk) * P].rearrange(
                                        "p (c f) -> p c f", f=P),
                                    in_ap=m_full[l][g][:],
                                    idxs_ap=idx16[:, cc * 8:(cc + kk) * 8],
                                    num_idxs=kk * 128,
                                    num_idxs_reg=kk * 128,
                                    elem_size=P,
                                    single_packet=False,
                                    queue_num=g)
                                cc += kk
                    # G == 1: per-chunk gathers are issued in the block loop
                    # right before their matmul (keeps SWDGE rings drained).
                    S = sp.tile([P, maxc * P], bf16, name="S", tag="S")
                    for b0 in range(0, k, SB):
                        nb = min(SB, k - b0)
                        g0c = c0 + b0
                        t_ = sp0.tile([P, SB * P], bf16, name="S0", tag="S0")
                        nc.vector.tensor_tensor(
                            out=t_[:, :nb * P].rearrange(
                                "p (c f) -> p c f", f=P),
                            in0=iotaB[:, :nb * P].rearrange(
                                "p (c f) -> p c f", f=P),
                            in1=eslot[:, g0c:g0c + nb].to_broadcast(
                                [P, nb, P]),
                            op=ALU.is_equal)
                        nc.vector.tensor_tensor(
                            out=S[:, b0 * P:(b0 + nb) * P].rearrange(
                                "p (c f) -> p c f", f=P),
                            in0=t_[:, :nb * P].rearrange(
                                "p (c f) -> p c f", f=P),
                            in1=ew[:, g0c:g0c + nb].to_broadcast([P, nb, P]),
                            op=ALU.mult)
                    slab_tiles[si] = (msg, S, c0)
                    return slab_tiles[si]

                for ci, (s, wdt, blks) in enumerate(chunks):
                    agg = aggp.tile([P, 512], f32r, name="agg", tag="agg")
                    for bi, j in enumerate(blks):
                        si = j // SLAB_J
                        msg, S, c0 = ensure_slab(si)
                        pj = pagg.tile([P, P], f32, name="pj", tag="agg128")
                        mm = []
                        for g in range(G):
                            for k in range(int(cap[j, g])):
                                mm.append(colbase[(j, g)] + k)
                        if not mm:
                            nc.vector.memset(pj[:], 0.0)
                        for ki, cc in enumerate(mm):
                            if G > 1:
                                lhs = msg[:, (cc - c0) * P:(cc - c0 + 1) * P]
                            else:
                                mt = msgp.tile([P, P], bf16, name="mt",
                                               tag="mt")
                                nc.gpsimd.indirect_dma_start(
                                    out=mt[:], out_offset=None,
                                    in_=m_full[l][:],
                                    in_offset=bass.IndirectOffsetOnAxis(
                                        ap=esrc[:, cc:cc + 1], axis=0))
                                lhs = mt[:]
                            nc.tensor.matmul(
                                pj[:],
                                lhsT=lhs,
                                rhs=S[:, (cc - c0) * P:(cc - c0 + 1) * P],
                                start=(ki == 0),
                                stop=(ki == len(mm) - 1))
                        nc.scalar.copy(out=agg[:, bi * P:(bi + 1) * P],
                                       in_=pj[:])

                    # ---- GRU for this chunk (f32r matmuls)
                    sl = slice(s, s + wdt)
                    pr = pbig.tile([P, 512], f32, name="pr", tag="big")
                    pz = pbig.tile([P, 512], f32, name="pz", tag="big")
                    pin = pbig.tile([P, 512], f32, name="pin", tag="big")
                    phn = pbig.tile([P, 512], f32, name="phn", tag="big")
                    for (ps_, g) in ((pr, 0), (pz, 1)):
                        gs = slice(g * P, (g + 1) * P)
                        nc.tensor.matmul(ps_[:, :wdt], lhsT=gWih[:, gs],
                                         rhs=agg[:, :wdt],
                                         start=True, stop=False)
                        nc.tensor.matmul(ps_[:, :wdt], lhsT=gWhh[:, gs],
                                         rhs=hT[:, sl],
                                         start=False, stop=True)
                    gn = slice(2 * P, 3 * P)
                    nc.tensor.matmul(pin[:, :wdt], lhsT=gWih[:, gn],
                                     rhs=agg[:, :wdt], start=True, stop=True)
                    nc.tensor.matmul(phn[:, :wdt], lhsT=gWhh[:, gn],
                                     rhs=hT[:, sl], start=True, stop=True)

                    rt = tp.tile([P, 512], f32, name="rt", tag="ew1")
                    zt = tp.tile([P, 512], f32, name="zt", tag="ew2")
                    t2 = tp.tile([P, 512], f32, name="t2", tag="ew3")
                    t3 = tp.tile([P, 512], f32, name="t3", tag="ew4")
                    nt = tp.tile([P, 512], f32, name="nt", tag="ew5")
                    dd = tp.tile([P, 512], f32, name="dd", tag="ew6")
                    ee = tp.tile([P, 512], f32, name="ee", tag="ew7")
                    nc.scalar.activation(rt[:, :wdt], pr[:, :wdt],
                                         AF.Sigmoid, bias=grub[:, 0:1])
                    nc.scalar.activation(zt[:, :wdt], pz[:, :wdt],
                                         AF.Sigmoid, bias=grub[:, 1:2])
                    nc.vector.scalar_tensor_tensor(
                        out=t2[:, :wdt], in0=phn[:, :wdt],
                        scalar=grub[:, 3:4], in1=rt[:, :wdt],
                        op0=ALU.add, op1=ALU.mult)
                    nc.vector.tensor_add(t3[:, :wdt], t2[:, :wdt],
                                         pin[:, :wdt])
                    nc.scalar.activation(nt[:, :wdt], t3[:, :wdt],
                                         AF.Tanh, bias=grub[:, 2:3])
                    hTf = hT[:, sl].bitcast(f32)
                    nc.vector.tensor_sub(dd[:, :wdt], hTf, nt[:, :wdt])
                    nc.vector.tensor_mul(ee[:, :wdt], zt[:, :wdt],
                                         dd[:, :wdt])
                    nc.vector.tensor_add(hT[:, sl], nt[:, :wdt],
                                         ee[:, :wdt])

                    if l < L - 1:
                        m_chunk_and_dma(l + 1, ci)
                    else:
                        # ---- LSTM for this chunk
                        hx = tp.tile([P, 512], bf16, name="hx", tag="ewx")
                        nc.vector.tensor_copy(hx[:, :wdt], hTf)
                        ht = tp.tile([P, 512], bf16, name="htc", tag="ewhl")
                        ct = tp.tile([P, 512], f32, name="ctc", tag="ewcl")
                        nc.sync.dma_start(ht[:, :wdt], HT_in[:, sl])
                        nc.sync.dma_start(ct[:, :wdt], CT_in[:, sl])
                        pg = [pbig.tile([P, 512], f32, name=f"pl{g}",
                                        tag="big") for g in range(4)]
                        for g in range(4):
                            gs = slice(g * P, (g + 1) * P)
                            nc.tensor.matmul(pg[g][:, :wdt],
                                             lhsT=lWih[:, gs],
                                             rhs=hx[:, :wdt], start=True,
                                             stop=False)
                            nc.tensor.matmul(pg[g][:, :wdt],
                                             lhsT=lWhh[:, gs],
                                             rhs=ht[:, :wdt], start=False,
                                             stop=True)
                        it = tp.tile([P, 512], f32, name="it", tag="ew1")
                        ft = tp.tile([P, 512], f32, name="ft", tag="ew2")
                        gt = tp.tile([P, 512], f32, name="gt", tag="ew3")
                        ot = tp.tile([P, 512], f32, name="ot", tag="ew4")
                        nc.scalar.activation(it[:, :wdt], pg[0][:, :wdt],
                                             AF.Sigmoid, bias=lstmb[:, 0:1])
                        nc.scalar.activation(ft[:, :wdt], pg[1][:, :wdt],
                                             AF.Sigmoid, bias=lstmb[:, 1:2])
                        nc.scalar.activation(gt[:, :wdt], pg[2][:, :wdt],
                                             AF.Tanh, bias=lstmb[:, 2:3])
                        nc.scalar.activation(ot[:, :wdt], pg[3][:, :wdt],
                                             AF.Sigmoid, bias=lstmb[:, 3:4])
                        t1 = tp.tile([P, 512], f32, name="lt1", tag="ew5")
                        t2b = tp.tile([P, 512], f32, name="lt2", tag="ew6")
                        cn = tp.tile([P, 512], f32, name="cn", tag="ew7")
                        tc_ = tp.tile([P, 512], f32, name="tcx", tag="ewt")
                        hn = tp.tile([P, 512], f32, name="hn", tag="ewh")
                        nc.vector.tensor_mul(t1[:, :wdt], ft[:, :wdt],
                                             ct[:, :wdt])
                        nc.vector.tensor_mul(t2b[:, :wdt], it[:, :wdt],
                                             gt[:, :wdt])
                        nc.vector.tensor_add(cn[:, :wdt], t1[:, :wdt],
                                             t2b[:, :wdt])
                        nc.scalar.activation(tc_[:, :wdt], cn[:, :wdt],
                                             AF.Tanh)
                        nc.vector.tensor_mul(hn[:, :wdt], ot[:, :wdt],
                                             tc_[:, :wdt])
                        nc.sync.dma_start(Cout_ext[:, sl], cn[:, :wdt])
                        nc.sync.dma_start(Hout_ext[:, sl], hn[:, :wdt])
    return nc


_CACHE = {}


def kernel(X, edge_index, edge_weight, H, C, conv_W,
           gru_Wih, gru_Whh, gru_bih, gru_bhh,
           lstm_Wih, lstm_Whh, lstm_bih, lstm_bhh):
    X = np.asarray(X, dtype=np.float32)
    H = np.asarray(H, dtype=np.float32)
    C = np.asarray(C, dtype=np.float32)
    conv_W = np.asarray(conv_W, dtype=np.float32)
    edge_index = np.asarray(edge_index)
    edge_weight = np.asarray(edge_weight, dtype=np.float32)

    N, D = X.shape
    L = conv_W.shape[0]
    assert D == P and N % NCORES == 0
    NL = N // NCORES
    NB = (NL + P - 1) // P
    NLP = NB * P

    mode = os.environ.get("GATHER_MODE", "ant")
    G = 4 if mode == "ant" else 1
    GR = (NL // 4) if G > 1 else N
    gather_n = int(os.environ.get("GATHER_N", "16"))

    src = edge_index[0].astype(np.int64)
    dst = edge_index[1].astype(np.int64)
    newpos = _balance_nodes(dst, N, NL, NB)
    perm = np.empty(N, dtype=np.int64)          # new id -> orig id
    perm[newpos] = np.arange(N)
    e_new = np.stack([newpos[src], newpos[dst]])

    edata, cap, ncols, slabs, colbase = _preprocess_edges(
        e_new, edge_weight, N, NL, NB, G, GR)

    key = (N, D, L, ncols, cap.tobytes(), G, gather_n)
    if key not in _CACHE:
        nc = _build(N, D, L, NL, NB, NLP, cap, ncols, slabs, colbase, G, GR,
                    gather_n)
        nc.compile()
        _CACHE[key] = nc
    nc = _CACHE[key]

    Xp, Hp, Cp = X[perm], H[perm], C[perm]

    gWihT = np.ascontiguousarray(np.asarray(gru_Wih, np.float32).T)
    gWhhT = np.ascontiguousarray(np.asarray(gru_Whh, np.float32).T)
    lWihT = np.ascontiguousarray(
        np.asarray(lstm_Wih, np.float32).T).astype(BF)
    lWhhT = np.ascontiguousarray(
        np.asarray(lstm_Whh, np.float32).T).astype(BF)
    gb = np.asarray(gru_bih, np.float32)
    gb2 = np.asarray(gru_bhh, np.float32)
    grub = np.stack([gb[0:D] + gb2[0:D], gb[D:2 * D] + gb2[D:2 * D],
                     gb[2 * D:3 * D], gb2[2 * D:3 * D]], axis=1)
    lb = np.asarray(lstm_bih, np.float32) + np.asarray(lstm_bhh, np.float32)
    lstmb = np.stack([lb[g * D:(g + 1) * D] for g in range(4)], axis=1)
    iotaB = np.ascontiguousarray(np.broadcast_to(
        np.arange(P, dtype=np.float32), (P, SB, P)).reshape(P, SB * P)
    ).astype(BF)
    convWb = np.ascontiguousarray(
        np.concatenate([conv_W[i] for i in range(L)], axis=1))

    in_maps = []
    for r in range(NCORES):
        sl = slice(r * NL, (r + 1) * NL)
        im = dict(
            hT0=_padT(Xp[sl], NLP),
            HT=_padT(Hp[sl], NLP, BF),
            CT=_padT(Cp[sl], NLP),
            convW=convWb, gWihT=gWihT, gWhhT=gWhhT, grub=grub,
            lWihT=lWihT, lWhhT=lWhhT, lstmb=lstmb,
            eslot=edata[r]['eslot'], ew=edata[r]['ew'],
            iotaB=iotaB,
        )
        if G > 1:
            im['idx16'] = edata[r]['idx16']
        else:
            im['esrc'] = edata[r]['esrc']
        in_maps.append(im)

    if os.environ.get("KERNEL_SIM"):
        from concourse import bass_interp
        simu = bass_interp.MultiCoreSim(nc, NCORES)
        for r in range(NCORES):
            for k, v in in_maps[r].items():
                simu.cores[r].tensor(k)[:] = v
        simu.simulate()
        results = [{k: np.asarray(simu.cores[r].mem_tensor(k))
                    for k in ("HoutT", "CoutT")} for r in range(NCORES)]
    else:
        trace = bool(int(os.environ.get("KERNEL_TRACE", "0")))
        res = run_bass_kernel_spmd(nc, in_maps, core_ids=list(range(NCORES)),
                                   trace=trace)
        if trace:
            kernel.last_exec_time_ns = res.exec_time_ns
        results = res.results

    Hnew = np.empty((N, D), dtype=np.float32)
    Cnew = np.empty((N, D), dtype=np.float32)
    for r in range(NCORES):
        sl = slice(r * NL, (r + 1) * NL)
        Hnew[sl] = results[r]["HoutT"].T[:NL]
        Cnew[sl] = results[r]["CoutT"].T[:NL]
    Hout = Hnew[newpos]
    Cout = Cnew[newpos]
    return Hout, Hout, Cout


kernel.last_exec_time_ns = None

